# revision 1
# baseline (speedup 1.0000x reference)
"""Stereo correlation cost volume kernel for Trainium2 (8 NeuronCores).

  out[b, d, h, w] = mean_c( L[b,c,h,w] * R[b,c,h,w-d] )  for w >= d, else 0
  B=8, C=64, H=128, W=256, D=64.

Sharding: data-parallel over batch; core b handles batch b.

Per-core algorithm (per h row):
  1. PE computes the Gram G2[u, w] = sum_c R[c,u] * L[c,w] (fp32 PSUM
     accumulation) for two u-blocks of 128, w in [0, 256).
  2. The Gram is copied PSUM->SBUF (DVE/ACT, cast to the scratch dtype)
     and DMA'd to a DRAM scratch as dense [128 x 256] row-major blocks.
  3. A second DMA re-reads the scratch through a (pitch+1)-strided view:
     element (p, d) at  p*(256+1) + off + d  ==  row p, col p+off+d,
     which materializes the skewed tile T3[u, d] = G2[u, u+d]
     (= out[d, u+d] * C) with contiguous inner runs.
  4. PE transposes T3 -> PT[d, u], DVE/ACT scale by 1/C (cast back to
     fp32) into SBUF, and a strided DMA writes out[d, h, w=u+d]
     (partition stride H*WP+1).
The output DRAM tensor is padded to WP=320 columns so the fixed-size
skew/write APs can spill harmlessly for w >= 256; the host slices w<256.
The runner pre-zeros output buffers, so the w < d triangle stays zero.
"""

import os
import sys

import numpy as np

sys.path.insert(0, "/opt/trn_rl_repo")

import ml_dtypes  # noqa: E402

import concourse.bass as bass  # noqa: E402
import concourse.bacc as bacc  # noqa: E402
import concourse.mybir as mybir  # noqa: E402
from concourse.bass import AP  # noqa: E402
from concourse.bass_utils import run_bass_kernel_spmd  # noqa: E402
from concourse.masks import make_identity  # noqa: E402
from concourse.tile import TileContext  # noqa: E402

B, C, H, W = 8, 64, 128, 256
D = 64
WP = 320  # padded output width
NH = 8  # h rows per group
NG = H // NH  # 16 groups
F32 = mybir.dt.float32
F16 = mybir.dt.float16

# compute/in/scratch dtype: "bf16" (fast) or "f32" (exact)
USE_BF16 = os.environ.get("CORVOL_F32", "") != "1"

# scratch layout (in elements of the scratch dtype), per h slot:
#   [A rows: 128 x 256][B rows: 128 x 128][slack: 64]
G_COLS = 256
B_COLS = 256
BLK = 128 * G_COLS  # 32768
BLKB = 128 * B_COLS
HSLOT = BLK + BLKB + 64
SCR_SLOTS = H // 2  # two scratch tensors alternate by group parity
SCR_SIZE = SCR_SLOTS * HSLOT

_CACHE = {}


def build():
    in_dt = F16 if USE_BF16 else F32
    nc = bacc.Bacc()
    lr_dram = nc.dram_tensor("lr", [C, H, 2, W], in_dt, kind="ExternalInput")
    out_dram = nc.dram_tensor("out", [D, H, WP], F32, kind="ExternalOutput")
    scr = [
        nc.dram_tensor(f"scratch{i}", [SCR_SIZE], in_dt, kind="Internal")
        for i in range(2)
    ]

    with TileContext(nc) as tc:
        with (
            tc.tile_pool(name="const", bufs=1) as pconst,
            tc.tile_pool(name="inp", bufs=2) as pin,
            tc.tile_pool(name="gband", bufs=3) as pg,
            tc.tile_pool(name="skew", bufs=3) as pt3,
            tc.tile_pool(name="outs", bufs=3) as ps8,
            tc.tile_pool(name="psA", bufs=2, space="PSUM") as ppa,
            tc.tile_pool(name="psB", bufs=2, space="PSUM") as ppb,
            tc.tile_pool(name="psTA", bufs=2, space="PSUM") as ppta,
            tc.tile_pool(name="psTB", bufs=2, space="PSUM") as pptb,
        ):
            ident = pconst.tile([128, 128], in_dt)
            make_identity(nc, ident)
            zeros = pconst.tile([64, 64], in_dt)
            nc.gpsimd.memset(zeros, 0.0)
            # zero the per-slot slack so skew-read spill never reads uninit
            for i in range(2):
                nc.sync.dma_start(
                    out=AP(scr[i], BLK + BLKB, [[HSLOT, 64], [1, 64]]),
                    in_=zeros[:, :],
                )
            # warmup: absorb the gpsimd ident-write wait on PE once, writing
            # into a g2a-pool slot (no dedicated psum bank needed)
            scrap0 = ppa.tile([64, 64], in_dt, tag="g2a")
            nc.tensor.transpose(
                scrap0[0:1, :], ident[0:64, 0:1], ident[0:64, 0:64]
            )

            pending = None
            for g in range(NG):
                h0 = g * NH
                sbase = (g // 2) * NH * HSLOT
                st = scr[g % 2]

                # stage 1: input load [SP]
                lr8 = pin.tile([C, NH * 2 * W], in_dt, tag="lr8")
                lr8v = lr8.rearrange("p (h t w) -> p h t w", h=NH, t=2)
                nc.sync.dma_start(out=lr8v, in_=lr_dram[:, h0 : h0 + NH, :, :])

                # stage 2: consume compute of previous group (PE/DVE/ACT
                # run these first in this window)
                s8avp = s8bvp = None
                if pending is not None:
                    t3vp = pending
                    s8a = ps8.tile([64, NH * 128], F32, tag="s8a")
                    s8b = ps8.tile([64, NH * 128], F32, tag="s8b")
                    s8avp = s8a.rearrange("p (h u) -> p h u", h=NH)
                    s8bvp = s8b.rearrange("p (h u) -> p h u", h=NH)
                    for hh in range(NH):
                        pta = ppta.tile([64, 128], in_dt, tag="pta")
                        ptb = pptb.tile([64, 128], in_dt, tag="ptb")
                        nc.tensor.transpose(pta, t3vp[:, hh, 0:64], ident)
                        nc.tensor.transpose(ptb, t3vp[:, hh, 64:128], ident)
                        nc.vector.tensor_scalar_mul(
                            s8avp[:, hh, :], pta, 1.0 / C
                        )
                        nc.scalar.mul(s8bvp[:, hh, :], ptb, 1.0 / C)

                # stage 3: produce compute for this group
                g8 = pg.tile([128, NH * (G_COLS + B_COLS)], in_dt, tag="g8")
                g8v = g8.rearrange("p (h c) -> p h c", h=NH)
                for hh in range(NH):
                    g2a = ppa.tile([128, G_COLS], F32, tag="g2a")
                    g2b = ppb.tile([128, B_COLS], F32, tag="g2b")
                    nc.tensor.matmul(
                        g2a,
                        lhsT=lr8v[:, hh, 1, 0:128],
                        rhs=lr8v[:, hh, 0, :],
                    )
                    nc.tensor.matmul(
                        g2b,
                        lhsT=lr8v[:, hh, 1, 128:256],
                        rhs=lr8v[:, hh, 0, :],
                    )
                    nc.vector.tensor_copy(g8v[:, hh, 0:G_COLS], g2a)
                    nc.scalar.copy(g8v[:, hh, G_COLS : G_COLS + B_COLS], g2b)

                # stage 4: out-DMAs of previous group [SP, before Gwrites]
                if s8avp is not None:
                    ph0 = (g - 1) * NH
                    nc.sync.dma_start(
                        out=AP(
                            out_dram,
                            ph0 * WP,
                            [[H * WP + 1, 64], [WP, NH], [1, 128]],
                        ),
                        in_=s8avp,
                    )
                    nc.sync.dma_start(
                        out=AP(
                            out_dram,
                            ph0 * WP + 128,
                            [[H * WP + 1, 64], [WP, NH], [1, 128]],
                        ),
                        in_=s8bvp,
                    )

                # stage 5: Gram -> scratch [SP]
                nc.sync.dma_start(
                    out=AP(st, sbase, [[G_COLS, 128], [HSLOT, NH], [1, G_COLS]]),
                    in_=g8v[:, :, 0:G_COLS],
                )
                nc.sync.dma_start(
                    out=AP(
                        st,
                        sbase + BLK,
                        [[B_COLS, 128], [HSLOT, NH], [1, B_COLS]],
                    ),
                    in_=g8v[:, :, G_COLS : G_COLS + B_COLS],
                )

                # stage 6: skewed re-read [SP]
                t3 = pt3.tile([128, NH * 128], in_dt, tag="t3")
                t3v = t3.rearrange("p (h d) -> p h d", h=NH)
                nc.sync.dma_start(
                    out=t3v[:, :, 0:64],
                    in_=AP(
                        st, sbase, [[G_COLS + 1, 128], [HSLOT, NH], [1, 64]]
                    ),
                )
                nc.sync.dma_start(
                    out=t3v[:, :, 64:128],
                    in_=AP(
                        st,
                        sbase + BLK + 128,
                        [[B_COLS + 1, 128], [HSLOT, NH], [1, 64]],
                    ),
                )
                pending = t3v

            # drain last group
            t3vp = pending
            s8a = ps8.tile([64, NH * 128], F32, tag="s8a")
            s8b = ps8.tile([64, NH * 128], F32, tag="s8b")
            s8av = s8a.rearrange("p (h u) -> p h u", h=NH)
            s8bv = s8b.rearrange("p (h u) -> p h u", h=NH)
            for hh in range(NH):
                pta = ppta.tile([64, 128], in_dt, tag="pta")
                ptb = pptb.tile([64, 128], in_dt, tag="ptb")
                nc.tensor.transpose(pta, t3vp[:, hh, 0:64], ident)
                nc.tensor.transpose(ptb, t3vp[:, hh, 64:128], ident)
                nc.vector.tensor_scalar_mul(s8av[:, hh, :], pta, 1.0 / C)
                nc.scalar.mul(s8bv[:, hh, :], ptb, 1.0 / C)
            ph0 = (NG - 1) * NH
            nc.sync.dma_start(
                out=AP(
                    out_dram,
                    ph0 * WP,
                    [[H * WP + 1, 64], [WP, NH], [1, 128]],
                ),
                in_=s8av,
            )
            nc.sync.dma_start(
                out=AP(
                    out_dram,
                    ph0 * WP + 128,
                    [[H * WP + 1, 64], [WP, NH], [1, 128]],
                ),
                in_=s8bv,
            )
    nc.finalize()
    return nc


def kernel(left_feature, right_feature, max_disp):
    assert int(max_disp) == D
    left = np.asarray(left_feature, dtype=np.float32)
    right = np.asarray(right_feature, dtype=np.float32)
    assert left.shape == (B, C, H, W) and right.shape == (B, C, H, W)

    if "nc" not in _CACHE:
        _CACHE["nc"] = build()
    nc = _CACHE["nc"]

    np_dt = np.float16 if USE_BF16 else np.float32
    in_maps = []
    for b in range(B):
        lr = np.ascontiguousarray(
            np.stack([left[b], right[b]], axis=2).astype(np_dt)
        )  # [C, H, 2, W]
        in_maps.append({"lr": lr})
    res = run_bass_kernel_spmd(nc, in_maps, list(range(B)))
    _CACHE["last_results"] = res
    out = np.stack([res.results[b]["out"][:, :, :W] for b in range(B)], axis=0)
    return out.astype(np.float32)



# revision 2
# speedup vs baseline: 1.1251x; 1.1251x over previous
"""Stereo correlation cost volume kernel for Trainium2 (8 NeuronCores).

  out[b, d, h, w] = mean_c( L[b,c,h,w] * R[b,c,h,w-d] )  for w >= d, else 0
  B=8, C=64, H=128, W=256, D=64.

Sharding: data-parallel over batch; core b handles batch b.

Only a 64-wide diagonal band of the Gram G[u, w] = sum_c R[c,u] L[c,w]
is ever needed (u = w - d, d in [0,64)), so instead of the full 256x256
Gram we compute four 64-row x 128-col band windows per h row:
  blk k: u in [64k, 64k+64), w in [64k, 64k+128)
(blk 3's w >= 256 half is filled with a harmless R.T@R product so the
scratch region is fully initialized; it only ever lands in the output
padding columns, which the host slices off.)

Per-core algorithm (per h row, pipelined over groups of NH=8 rows):
  1. PE computes blk0/blk1 into one 128-partition PSUM tile (blk1 at
     partition offset 64) and blk2/blk3 into a second; DVE/ACT copy the
     pair tiles to SBUF as f16, h-major: g8[p, k, h, j].
  2. One DMA per pair writes DRAM scratch rows r = p%64 at
     addr = blk_base + r*1024 + h*128 + j  (p-linear, 2 KB runs).
  3. Four skew-read DMAs re-read each blk through a (1025)-strided view:
     addr = blk_base + r*1025 + h*128 + d  ==  row r, col r+d, i.e. the
     diagonal tile T[u, d] = G[u, u+d] with contiguous 64-elem d-runs.
  4. PE transposes the two 128-partition skew tiles -> PT[d, u-span],
     DVE/ACT scale by 1/C into an f16 row tile s8[d, h, 0:256], and one
     DMA writes out[d, h, w = u + d] (partition stride H*WP+1).
The output DRAM tensor is f16 and padded to WP=320 columns so the fixed
256-wide skewed writes spill harmlessly for w >= 256; the host slices
w < 256 and casts to f32. The runner pre-zeros output buffers, so the
w < d triangle stays zero.
"""

import os
import sys

import numpy as np

sys.path.insert(0, "/opt/trn_rl_repo")

import ml_dtypes  # noqa: E402

import concourse.bass as bass  # noqa: E402
import concourse.bacc as bacc  # noqa: E402
import concourse.mybir as mybir  # noqa: E402
from concourse.bass import AP  # noqa: E402
from concourse.bass_utils import run_bass_kernel_spmd  # noqa: E402
from concourse.masks import make_identity  # noqa: E402
from concourse.tile import TileContext  # noqa: E402

B, C, H, W = 8, 64, 128, 256
D = 64
WP = 320  # padded output width
NH = 8  # h rows per group
NG = H // NH  # 16 groups
F32 = mybir.dt.float32
F16 = mybir.dt.float16

# compute/in/scratch dtype: "bf16" (fast) or "f32" (exact-ish)
USE_BF16 = os.environ.get("CORVOL_F32", "") != "1"

# scratch layout (elements of the scratch dtype), per group:
#   4 band blocks, each 64 rows x (NH*128) cols row-major
GW = NH * 128  # 1024: scratch row width (h-major within a block row)
BLK = 64 * GW  # 65536 elements per band block
GRP = 4 * BLK  # 262144 elements per group
SCR_SIZE = (NG // 2) * GRP  # two scratch tensors alternate by group parity

_CACHE = {}


def build():
    in_dt = F16 if USE_BF16 else F32
    nc = bacc.Bacc()
    lr_dram = nc.dram_tensor("lr", [C, H, 2, W], in_dt, kind="ExternalInput")
    out_dram = nc.dram_tensor("out", [D, H, WP], in_dt, kind="ExternalOutput")
    scr = [
        nc.dram_tensor(f"scratch{i}", [SCR_SIZE], in_dt, kind="Internal")
        for i in range(2)
    ]

    with TileContext(nc) as tc:
        with (
            tc.tile_pool(name="const", bufs=1) as pconst,
            tc.tile_pool(name="inp", bufs=2) as pin,
            tc.tile_pool(name="gband", bufs=3) as pg,
            tc.tile_pool(name="skew", bufs=3) as pt3,
            tc.tile_pool(name="outs", bufs=3) as ps8,
            tc.tile_pool(name="psA", bufs=2, space="PSUM") as ppa,
            tc.tile_pool(name="psB", bufs=2, space="PSUM") as ppb,
            tc.tile_pool(name="psTA", bufs=2, space="PSUM") as ppta,
            tc.tile_pool(name="psTB", bufs=2, space="PSUM") as pptb,
        ):
            ident = pconst.tile([128, 128], in_dt)
            make_identity(nc, ident)
            # warmup: absorb the gpsimd ident-write wait on PE once
            scrap0 = ppa.tile([64, 64], in_dt, tag="g2a")
            nc.tensor.transpose(
                scrap0[0:1, :], ident[0:64, 0:1], ident[0:64, 0:64]
            )

            pending = None
            for g in range(NG):
                h0 = g * NH
                sbase = (g // 2) * GRP
                st = scr[g % 2]

                # stage 1: input load [SP]
                lr8 = pin.tile([C, NH * 2 * W], in_dt, tag="lr8")
                lr8v = lr8.rearrange("p (h t w) -> p h t w", h=NH, t=2)
                lr8f = lr8.rearrange("p (h x) -> p h x", h=NH)
                nc.sync.dma_start(out=lr8v, in_=lr_dram[:, h0 : h0 + NH, :, :])

                # stage 2: consume compute of previous group (PE/DVE/ACT
                # run these first in this window)
                s8vp = None
                if pending is not None:
                    t01p, t23p = pending
                    s8 = ps8.tile([64, NH * 256], in_dt, tag="s8")
                    s8vp = s8.rearrange("p (h u) -> p h u", h=NH)
                    for hh in range(NH):
                        pt1 = ppta.tile([64, 128], in_dt, tag="pta")
                        pt2 = pptb.tile([64, 128], in_dt, tag="ptb")
                        nc.tensor.transpose(pt1, t01p[:, hh, :], ident)
                        nc.tensor.transpose(pt2, t23p[:, hh, :], ident)
                        nc.vector.tensor_scalar_mul(
                            s8vp[:, hh, 0:128], pt1, 1.0 / C
                        )
                        nc.scalar.mul(s8vp[:, hh, 128:256], pt2, 1.0 / C)

                # stage 3: produce band blocks for this group
                #   blk0: u [0,64)    w [0,128)     -> pA parts 0:64
                #   blk1: u [64,128)  w [64,192)    -> pA parts 64:128
                #   blk2: u [128,192) w [128,256)   -> pB parts 0:64
                #   blk3: u [192,256) w [192,320)   -> pB parts 64:128
                # lr8f per h: cols [0,256) = L, [256,512) = R; blk3's rhs
                # spills into R cols -> finite garbage, lands in out pad.
                g8 = pg.tile([128, 2 * NH * 128], in_dt, tag="g8")
                g8v = g8.rearrange("p (k h j) -> p k h j", k=2, h=NH)
                for hh in range(NH):
                    pA = ppa.tile([128, 128], F32, tag="g2a")
                    pB = ppb.tile([128, 128], F32, tag="g2b")
                    nc.tensor.matmul(
                        pA[0:64, :],
                        lhsT=lr8v[:, hh, 1, 0:64],
                        rhs=lr8f[:, hh, 0:128],
                    )
                    nc.tensor.matmul(
                        pA[64:128, :],
                        lhsT=lr8v[:, hh, 1, 64:128],
                        rhs=lr8f[:, hh, 64:192],
                    )
                    nc.tensor.matmul(
                        pB[0:64, :],
                        lhsT=lr8v[:, hh, 1, 128:192],
                        rhs=lr8f[:, hh, 128:256],
                    )
                    nc.tensor.matmul(
                        pB[64:128, :],
                        lhsT=lr8v[:, hh, 1, 192:256],
                        rhs=lr8f[:, hh, 192:320],
                    )
                    nc.vector.tensor_copy(g8v[:, 0, hh, :], pA)
                    nc.scalar.copy(g8v[:, 1, hh, :], pB)

                # stage 4: out-DMA of previous group [SP, before band writes]
                if s8vp is not None:
                    ph0 = (g - 1) * NH
                    nc.sync.dma_start(
                        out=AP(
                            out_dram,
                            ph0 * WP,
                            [[H * WP + 1, 64], [WP, NH], [1, 256]],
                        ),
                        in_=s8vp,
                    )

                # stage 5: band blocks -> scratch [SP]; partition p of pair
                # tile k maps to blk (2k + p//64), row p%64 (p-linear).
                nc.sync.dma_start(
                    out=AP(st, sbase, [[GW, 128], [128, NH], [1, 128]]),
                    in_=g8v[:, 0, :, :],
                )
                nc.sync.dma_start(
                    out=AP(
                        st, sbase + 2 * BLK, [[GW, 128], [128, NH], [1, 128]]
                    ),
                    in_=g8v[:, 1, :, :],
                )

                # stage 6: skewed re-read [SP]
                t01 = pt3.tile([128, NH * 64], in_dt, tag="t01")
                t23 = pt3.tile([128, NH * 64], in_dt, tag="t23")
                t01v = t01.rearrange("p (h d) -> p h d", h=NH)
                t23v = t23.rearrange("p (h d) -> p h d", h=NH)
                for k in range(4):
                    dest = (t01v, t23v)[k // 2]
                    pr = (k % 2) * 64
                    nc.sync.dma_start(
                        out=dest[pr : pr + 64, :, :],
                        in_=AP(
                            st,
                            sbase + k * BLK,
                            [[GW + 1, 64], [128, NH], [1, 64]],
                        ),
                    )
                pending = (t01v, t23v)

            # drain last group
            t01p, t23p = pending
            s8 = ps8.tile([64, NH * 256], in_dt, tag="s8")
            s8v = s8.rearrange("p (h u) -> p h u", h=NH)
            for hh in range(NH):
                pt1 = ppta.tile([64, 128], in_dt, tag="pta")
                pt2 = pptb.tile([64, 128], in_dt, tag="ptb")
                nc.tensor.transpose(pt1, t01p[:, hh, :], ident)
                nc.tensor.transpose(pt2, t23p[:, hh, :], ident)
                nc.vector.tensor_scalar_mul(s8v[:, hh, 0:128], pt1, 1.0 / C)
                nc.scalar.mul(s8v[:, hh, 128:256], pt2, 1.0 / C)
            ph0 = (NG - 1) * NH
            nc.sync.dma_start(
                out=AP(
                    out_dram,
                    ph0 * WP,
                    [[H * WP + 1, 64], [WP, NH], [1, 256]],
                ),
                in_=s8v,
            )
    nc.finalize()
    return nc


def kernel(left_feature, right_feature, max_disp):
    assert int(max_disp) == D
    left = np.asarray(left_feature, dtype=np.float32)
    right = np.asarray(right_feature, dtype=np.float32)
    assert left.shape == (B, C, H, W) and right.shape == (B, C, H, W)

    if "nc" not in _CACHE:
        _CACHE["nc"] = build()
    nc = _CACHE["nc"]

    np_dt = np.float16 if USE_BF16 else np.float32
    in_maps = []
    for b in range(B):
        lr = np.ascontiguousarray(
            np.stack([left[b], right[b]], axis=2).astype(np_dt)
        )  # [C, H, 2, W]
        in_maps.append({"lr": lr})
    res = run_bass_kernel_spmd(nc, in_maps, list(range(B)))
    _CACHE["last_results"] = res
    out = np.stack(
        [res.results[b]["out"][:, :, :W] for b in range(B)], axis=0
    )
    return out.astype(np.float32)
